# revision 1
# baseline (speedup 1.0000x reference)
"""Trainium2 Bass kernel for nn_BasicBlock (gnn_message_passing).

Computation (reference):
    out = gelu(ln2(conv2(gelu(ln1(conv1(feats))))) + feats)
where conv(x) = einsum('nkc,kcd->nd', where(mask, x[nbr], 0), W).

Distribution: points (N) sharded across 8 cores; weights replicated; the
conv2 gather needs the full intermediate, so cores AllGather it between
stages (chunked, overlapped with conv1 compute).

Data path: gathers and matmuls run in fp16 (fp32 PSUM accumulation and
fp32 LayerNorm statistics), because the DMA xbar transpose — which turns
row-gathered [pts, cin] tiles into the [cin, pts] layout the PE needs —
only supports 2-byte dtypes. The neighbor mask is folded into the gather
indices host-side: masked slots point at an appended all-zero row.

LayerNorm rstd uses the scalar-engine Sqrt, but batched over LN_G tiles
(stats packed into one [P, 2*LN_G] tile) so the sqrt<->gelu activation
table switches amortize to 2 per group instead of 2 per tile. PSUM is
evacuated to SBUF fp16 right after the stats, freeing banks early.
"""

import os
import sys
from contextlib import ExitStack

import numpy as np

sys.path.insert(0, "/opt/trn_rl_repo")

import concourse.bass as bass
import concourse.mybir as mybir
import concourse.tile as tile
from concourse import bacc
from concourse.bass import ds
from concourse.bass_utils import run_bass_kernel_spmd

F16 = mybir.dt.float16
F32 = mybir.dt.float32
I32 = mybir.dt.int32
AF = mybir.ActivationFunctionType
ALU = mybir.AluOpType

N, K, C = 100000, 9, 256
CORES = 8
P = 128
EPS = 1e-6
TB = 2            # point-tiles per gather batch
AG_CHUNKS = 4     # conv1->conv2 AllGather chunks (overlap with conv1)
AG_LAG = 1        # tiles of gather-stream lag before issuing a collective
LN_G = 12         # tiles per LayerNorm sqrt group (amortizes act-table loads)
NZ = 128          # zero rows (spread masked-slot gathers across HBM banks)


def build_program(n_total, gamma1_trivial, beta1_trivial, gamma2_trivial,
                  beta2_trivial, tb=TB, cores=CORES, debug_dumps=False,
                  sim_no_collective=False, ag_chunks=AG_CHUNKS,
                  skip_collective=False):
    shard = n_total // cores
    tiles = (shard + P - 1) // P
    nsrc = n_total + NZ  # gather source rows incl. trailing zero rows

    nc = bacc.Bacc("TRN2", target_bir_lowering=False, debug=False,
                   num_devices=1 if sim_no_collective else cores)

    feats16 = nc.dram_tensor("feats16", [nsrc, C], F16, kind="ExternalInput")
    midx = nc.dram_tensor("midx", [P, tiles * K], I32, kind="ExternalInput")
    midx2 = nc.dram_tensor("midx2", [P, tiles * K], I32, kind="ExternalInput")
    w1 = nc.dram_tensor("w1", [P, K * 2 * C], F16, kind="ExternalInput")
    w2 = nc.dram_tensor("w2", [P, K * 2 * C], F16, kind="ExternalInput")
    res = nc.dram_tensor("res", [shard, C], F32, kind="ExternalInput")
    gb = nc.dram_tensor("gb", [4, C], F32, kind="ExternalInput")
    out = nc.dram_tensor("out", [shard, C], F32, kind="ExternalOutput")

    mid_shard = nc.dram_tensor("mid_shard", [shard, C], F16)
    mid_full = nc.dram_tensor("mid_full", [n_total + NZ, C], F16,
                              addr_space="Local" if sim_no_collective else "Shared")

    trivial1 = gamma1_trivial and beta1_trivial
    trivial2 = gamma2_trivial and beta2_trivial

    # AllGather chunk boundaries, in tiles.
    base_ct = tiles // ag_chunks
    rem = tiles % ag_chunks
    chunk_tiles = [base_ct + (1 if q < rem else 0) for q in range(ag_chunks)]
    chunk_t0 = [sum(chunk_tiles[:q]) for q in range(ag_chunks)]

    with ExitStack() as ctx:
        tc = ctx.enter_context(tile.TileContext(nc))
        singles = ctx.enter_context(tc.tile_pool(name="singles", bufs=1))
        gpool = ctx.enter_context(tc.tile_pool(name="gather", bufs=14))
        tpool = ctx.enter_context(tc.tile_pool(name="gt", bufs=10))
        mpool = ctx.enter_context(tc.tile_pool(name="misc", bufs=6))
        spool = ctx.enter_context(tc.tile_pool(name="stats", bufs=16))
        xpool = ctx.enter_context(tc.tile_pool(name="xb", bufs=LN_G + 6))
        psum = ctx.enter_context(tc.tile_pool(name="psum", bufs=6, space="PSUM"))

        w1_sb = singles.tile([P, K * 2 * C], F16)
        nc.sync.dma_start(out=w1_sb[:], in_=w1[:, :])
        w2_sb = singles.tile([P, K * 2 * C], F16)
        nc.sync.dma_start(out=w2_sb[:], in_=w2[:, :])
        idx_sb = singles.tile([P, tiles * K], I32)
        nc.sync.dma_start(out=idx_sb[:], in_=midx[:, :])
        idx2_sb = singles.tile([P, tiles * K], I32)
        nc.sync.dma_start(out=idx2_sb[:], in_=midx2[:, :])

        zblk = singles.tile([P, C], F16)
        nc.vector.memset(zblk[:], 0.0)
        # zero rows of the conv2 gather table (independent of the
        # collectives; write them up front)
        for zi in range(NZ // P):
            nc.sync.dma_start(
                out=mid_full[ds(n_total + zi * P, P), :], in_=zblk[:])

        nzero_sb = singles.tile([P, LN_G], F32)
        nc.vector.memset(nzero_sb[:], 0.0)
        eps_sb = singles.tile([P, 1], F32)
        nc.vector.memset(eps_sb[:], EPS)

        def bcast_row(row):
            t = singles.tile([P, C], F32)
            src = bass.AP(tensor=gb[:, :].tensor, offset=row * C,
                          ap=[[0, P], [1, C]])
            nc.gpsimd.dma_start(out=t[:], in_=src)
            return t

        g1b = None if gamma1_trivial else bcast_row(0)
        b1b = None if beta1_trivial else bcast_row(1)
        g2b = None if gamma2_trivial else bcast_row(2)
        b2b = None if beta2_trivial else bcast_row(3)

        def tt(out, in0, in1, op):
            nc.vector.tensor_tensor(out=out, in0=in0, in1=in1, op=op)

        def stride2_view(ap, col0, g):
            """[P, g] view of every second column of a [P, >=2g] AP."""
            return bass.AP(tensor=ap.tensor, offset=ap.offset + col0,
                           ap=[[ap.ap[0][0], ap.ap[0][1]], [2, g]])

        def ag_chunk(q):
            """AllGather chunk q of mid_shard into mid_full.

            mid_full uses a chunk-major layout (chunk q holds all cores'
            chunk-q rows contiguously, rank-major) so each chunked
            AllGather's output region is contiguous; conv2's gather
            indices are host-remapped to match."""
            r0 = chunk_t0[q] * P
            r1 = min((chunk_t0[q] + chunk_tiles[q]) * P, shard)
            rows = r1 - r0
            base = cores * r0  # sum of full chunk sizes before q, per layout
            if skip_collective:
                return
            if sim_no_collective:
                # on gpsimd to mimic the real collective's queue placement
                for peer in range(cores):
                    nc.gpsimd.dma_start(
                        out=mid_full[ds(base + peer * rows, rows), :],
                        in_=mid_shard[ds(r0, rows), :])
            else:
                nc.gpsimd.collective_compute(
                    "AllGather", ALU.bypass,
                    replica_groups=[list(range(cores))],
                    ins=[mid_shard[ds(r0, rows), :]],
                    outs=[mid_full[ds(base + 0, cores * rows), :]],
                )

        def conv_stage(src_dram, w_sb, stage):
            stage_idx = idx_sb if stage == 1 else idx2_sb
            ag_emitted = 0
            group = []   # [(t, xb_tile)] evacuated tiles awaiting LN finalize
            mvb = None   # [P, 2*LN_G] packed (mean, var) columns

            def finalize_group():
                nonlocal group, mvb
                if not group:
                    return
                g = len(group)
                # sqrt(x + eps) over all packed columns (mean columns give
                # garbage that is never read); one act-table episode per group
                sq = spool.tile([P, 2 * LN_G], F32, tag="sq")
                nc.scalar.activation(sq[:, :2 * g], mvb[:, :2 * g], AF.Sqrt,
                                     bias=eps_sb[:], scale=1.0)
                rs = spool.tile([P, 2 * LN_G], F32, tag="rs")
                nc.vector.reciprocal(rs[:, :2 * g], sq[:, :2 * g])
                nb = spool.tile([P, LN_G], F32, tag="nb")
                tt(nb[:, :g], stride2_view(mvb[:], 0, g),
                   stride2_view(rs[:], 1, g), ALU.mult)
                tt(nb[:, :g], nzero_sb[:, :g], nb[:, :g], ALU.subtract)
                for i, (t, xb) in enumerate(group):
                    rows = min(P, shard - t * P)
                    rstd = rs[:, 2 * i + 1:2 * i + 2]
                    nbias = nb[:, i:i + 1]
                    if stage == 1:
                        mt = mpool.tile([P, C], F16, tag="mid")
                        if trivial1:
                            nc.scalar.activation(mt[:], xb[:], AF.Gelu,
                                                 bias=nbias, scale=rstd)
                        else:
                            z = mpool.tile([P, C], F32, tag="z1")
                            nc.scalar.activation(z[:], xb[:], AF.Identity,
                                                 bias=nbias, scale=rstd)
                            if g1b is not None:
                                nc.vector.tensor_mul(z[:], z[:], g1b[:])
                            if b1b is not None:
                                nc.vector.tensor_add(z[:], z[:], b1b[:])
                            nc.scalar.activation(mt[:], z[:], AF.Gelu)
                        # writes issue from the Act queue: they depend on the
                        # act anyway, and keep the SP queue a pure transpose
                        # stream (no head-of-line blocking of the pipeline)
                        nc.scalar.dma_start(
                            out=mid_shard[ds(t * P, rows), :],
                            in_=mt[:rows, :])
                    else:
                        rt = mpool.tile([P, C], F32, tag="res")
                        nc.scalar.dma_start(out=rt[:rows, :],
                                            in_=res[ds(t * P, rows), :])
                        z = mpool.tile([P, C], F32, tag="z2")
                        nc.scalar.activation(z[:], xb[:], AF.Identity,
                                             bias=nbias, scale=rstd)
                        if g2b is not None:
                            nc.vector.tensor_mul(z[:], z[:], g2b[:])
                        if b2b is not None:
                            nc.vector.tensor_add(z[:], z[:], b2b[:])
                        so = mpool.tile([P, C], F32, tag="s2")
                        nc.vector.tensor_add(so[:], z[:], rt[:])
                        oo = mpool.tile([P, C], F32, tag="o2")
                        nc.scalar.activation(oo[:], so[:], AF.Gelu)
                        nc.scalar.dma_start(out=out[ds(t * P, rows), :],
                                            in_=oo[:rows, :])
                group = []
                mvb = None

            for t in range(tiles):
                if stage == 1:
                    # Emit pending chunk collectives with AG_LAG tiles of
                    # slack so the Pool queue never stalls on mid writes.
                    while (ag_emitted < ag_chunks - 1 and
                           t >= chunk_t0[ag_emitted] + chunk_tiles[ag_emitted]
                           + AG_LAG):
                        finalize_group()  # mid writes must reach the chunk
                        ag_chunk(ag_emitted)
                        ag_emitted += 1
                g_rows = gpool.tile([P, K, C], F16, tag="g_rows")
                for j in range(K):
                    # one gathered row per partition — the only indirect
                    # form the DGE ucode executes correctly
                    nc.gpsimd.indirect_dma_start(
                        out=g_rows[:, j, :],
                        out_offset=None,
                        in_=src_dram[:, :],
                        in_offset=bass.IndirectOffsetOnAxis(
                            ap=stage_idx[:, ds(t * K + j, 1)], axis=0),
                    )
                # [pts, K*C] -> 18 slabs of [cin_half, pts]
                gt = tpool.tile([P, 2 * K, P], F16, tag="gt")
                nc.sync.dma_start_transpose(out=gt[:], in_=g_rows[:, :, :])
                ps = psum.tile([P, C], F32, tag="ps")
                for k in range(K):
                    for h in range(2):
                        s = 2 * k + h
                        nc.tensor.matmul(
                            ps[:],
                            lhsT=gt[:, s, :],
                            rhs=w_sb[:, ds(s * C, C)],
                            start=(s == 0),
                            stop=(s == 2 * K - 1),
                        )
                # stats + evacuate psum to SBUF f16 so the bank frees
                # immediately; LN finalization happens per group
                if mvb is None:
                    mvb = spool.tile([P, 2 * LN_G], F32, tag="mvb")
                gi = len(group)
                st6 = spool.tile([P, 6], F32, tag="st6")
                nc.vector.bn_stats(st6[:], ps[:])
                nc.vector.bn_aggr(mvb[:, 2 * gi:2 * gi + 2], st6[:])
                xb = xpool.tile([P, C], F16, tag="xb")
                nc.vector.tensor_copy(xb[:], ps[:])
                group.append((t, xb))
                if len(group) == LN_G:
                    finalize_group()
            finalize_group()
            if stage == 1:
                while ag_emitted < ag_chunks:
                    ag_chunk(ag_emitted)
                    ag_emitted += 1

        conv_stage(feats16, w1_sb, 1)
        conv_stage(mid_full, w2_sb, 2)

    nc.compile()
    return nc


def prep_inputs(inputs, cores=CORES):
    """Host-side shard/layout prep (numpy only)."""
    feats = np.ascontiguousarray(np.asarray(inputs["feats"], dtype=np.float32))
    w1 = np.asarray(inputs["W1"], dtype=np.float32)
    w2 = np.asarray(inputs["W2"], dtype=np.float32)
    gamma1 = np.asarray(inputs["gamma1"], dtype=np.float32)
    beta1 = np.asarray(inputs["beta1"], dtype=np.float32)
    gamma2 = np.asarray(inputs["gamma2"], dtype=np.float32)
    beta2 = np.asarray(inputs["beta2"], dtype=np.float32)
    nbr = np.asarray(inputs["neighbor_idx"], dtype=np.int32)
    mask = np.asarray(inputs["neighbor_mask"])

    n, c = feats.shape
    k = nbr.shape[1]
    shard = n // cores
    tiles = (shard + P - 1) // P
    shard_pad = tiles * P

    # masked gather indices: masked/pad slots spread across NZ zero rows
    # (a single zero row is an HBM hotspot - same-address descriptors
    # serialize on one channel)
    zr = n + (np.arange(mask.size, dtype=np.int64) % NZ).reshape(mask.shape)
    midx = np.where(mask, nbr, zr).astype(np.int32)

    # conv2 gathers from mid_full, which uses a chunk-major layout (see
    # ag_chunk): row m = (owner c, local i) lands at cores*r0_q + c*R_q +
    # (i - r0_q) for its chunk q. Build the remap for all n+1 rows.
    base_ct = tiles // AG_CHUNKS
    rem_ct = tiles % AG_CHUNKS
    c_tiles = [base_ct + (1 if q < rem_ct else 0) for q in range(AG_CHUNKS)]
    c_t0 = [sum(c_tiles[:q]) for q in range(AG_CHUNKS)]
    remap = np.empty(n + NZ, np.int32)
    remap[n:] = np.arange(n, n + NZ, dtype=np.int32)  # zero rows keep slots
    loc = np.arange(shard, dtype=np.int64)
    for ci in range(cores):
        r_all = np.arange(ci * shard, (ci + 1) * shard, dtype=np.int64)
        for q in range(AG_CHUNKS):
            r0 = c_t0[q] * P
            r1 = min((c_t0[q] + c_tiles[q]) * P, shard)
            rows_q = r1 - r0
            remap[r_all[r0:r1]] = (cores * r0 + ci * rows_q
                                   + (loc[r0:r1] - r0)).astype(np.int32)
    midx2 = remap[midx]

    feats16 = np.vstack([feats, np.zeros((NZ, c), np.float32)]).astype(np.float16)

    def w_layout(w):
        # w1_sb[p, (2k+h)*C + d] = W[k, h*128+p, d]
        return np.ascontiguousarray(
            w.reshape(k, 2, P, c).transpose(2, 0, 1, 3).reshape(P, k * 2 * c)
        ).astype(np.float16)

    w1_t = w_layout(w1)
    w2_t = w_layout(w2)
    gbmat = np.stack([gamma1, beta1, gamma2, beta2]).astype(np.float32)

    in_maps = []
    for ci in range(cores):
        rows = slice(ci * shard, (ci + 1) * shard)
        mi = midx[rows]
        if shard_pad > shard:
            padz = n + (np.arange((shard_pad - shard) * k) % NZ).reshape(
                shard_pad - shard, k).astype(np.int32)
            mi = np.vstack([mi, padz])
        midx_t = np.ascontiguousarray(
            mi.reshape(tiles, P, k).transpose(1, 0, 2).reshape(P, tiles * k))
        mi2 = midx2[rows]
        if shard_pad > shard:
            mi2 = np.vstack([mi2, padz])
        midx2_t = np.ascontiguousarray(
            mi2.reshape(tiles, P, k).transpose(1, 0, 2).reshape(P, tiles * k))
        in_maps.append({
            "feats16": feats16,
            "midx": midx_t,
            "midx2": midx2_t,
            "w1": w1_t,
            "w2": w2_t,
            "res": np.ascontiguousarray(feats[rows]),
            "gb": gbmat,
        })

    flags = (
        bool(np.all(gamma1 == 1.0)), bool(np.all(beta1 == 0.0)),
        bool(np.all(gamma2 == 1.0)), bool(np.all(beta2 == 0.0)),
    )
    return in_maps, flags, n


def run(inputs, trace=False, cores=CORES, trace_kwargs=None):
    in_maps, flags, n = prep_inputs(inputs, cores=cores)
    nc = build_program(n, *flags, cores=cores)
    r = run_bass_kernel_spmd(nc, in_maps, core_ids=list(range(cores)),
                             trace=trace, **(trace_kwargs or {}))
    out = np.concatenate([r.results[ci]["out"] for ci in range(cores)], axis=0)
    return np.ascontiguousarray(out[:n]).astype(np.float32), r


def kernel(**inputs):
    out, _ = run(inputs, trace=False)
    return out

